# revision 18
# baseline (speedup 1.0000x reference)
"""Location-sensitive (Bahdanau) attention on 8 TRN2 NeuronCores.

Data-parallel: batch B=32 sharded 4-per-core; params replicated.

Per core, per batch b:
  E^T(c,t) = sum_h V[c,h] * enc[t,h]          (bf16 matmuls, c on partitions)
           + sum_j M[c,j] * a_shift[j,t]      (K=3 matmul; M = U @ conv_w fold)
  tanh with fused bias qb[c] = (dec @ W^T)[c] + b[c] + (U @ conv_b)[c]
  scores[t] = sum_c w[c] * tanh(...)          (K=128 matmuls, lhsT = w column)
  alignment = softmax(scores)                 (f32, on-chip)
  context[h] = sum_t alignment[t] * enc[t,h]  (DVE fused mul+reduce vs resident encT)
"""

import numpy as np
import ml_dtypes

import concourse.bass as bass
import concourse.mybir as mybir
import concourse.tile as tile
from concourse import bacc
from concourse.bass_utils import run_bass_kernel_spmd

B, T, H, CTX = 32, 1536, 1024, 1024
NCORES = 8
BL = B // NCORES          # 4 batches per core
P = 128
HT = H // P               # 8 h-tiles (contraction)
CT = CTX // P             # 8 c-tiles (output partitions)
NT = 512                  # t-tile (matmul free dim)
TT = T // NT              # 3 t-tiles
BF = mybir.dt.bfloat16
F32 = mybir.dt.float32
BF_NP = ml_dtypes.bfloat16
AX = mybir.AxisListType
AF = mybir.ActivationFunctionType
ALU = mybir.AluOpType

_CACHE = {}
import os as _os
_DISABLED = set(_os.environ.get("KPROBE", "").split(","))
_STAGE = int(_os.environ.get("KSTAGE", "4"))


def _emit(nc, d):
    """Emit the whole per-core program under a TileContext."""
    with tile.TileContext(nc) as tc:
        with (
            tc.tile_pool(name="params", bufs=1) as params,
            tc.tile_pool(name="enc", bufs=3) as encp,
            tc.tile_pool(name="ash", bufs=2) as ashp,
            tc.tile_pool(name="work", bufs=4) as work,
            tc.tile_pool(name="rows", bufs=2) as rows,
            tc.tile_pool(name="bc", bufs=2) as bcp,
            tc.tile_pool(name="psE", bufs=2, space="PSUM") as psE,
            tc.tile_pool(name="psS", bufs=2, space="PSUM") as psS,
            tc.tile_pool(name="psQ", bufs=2, space="PSUM") as psQ,
            tc.tile_pool(name="psB", bufs=2, space="PSUM") as psB,
        ):
            # ---- resident params ----
            VT_sb = params.tile([P, HT, CTX], BF)
            WT_sb = params.tile([P, HT, CTX], BF)
            MT_sb = params.tile([P, CTX], BF)      # rows 3..127 zero (K padded)
            wre_sb = params.tile([P, CT], BF)
            biascol_sb = params.tile([P, CT], F32)
            decT_sb = params.tile([P, HT, BL], BF)
            qb_sb = params.tile([P, CT, BL], F32)
            bc_lhs = params.tile([P, P], BF)       # row 0 ones, rest zero

            for hi in range(HT):
                nc.sync.dma_start(
                    VT_sb[:, hi, :],
                    d["VT"].ap().rearrange("(o p) c -> p o c", p=P)[:, hi, :],
                )
                nc.sync.dma_start(
                    WT_sb[:, hi, :],
                    d["WT"].ap().rearrange("(o p) c -> p o c", p=P)[:, hi, :],
                )
            nc.vector.memset(MT_sb[:], 0.0)
            nc.sync.dma_start(MT_sb[0:3, :], d["MT"].ap())
            nc.sync.dma_start(wre_sb[:], d["wre"].ap())
            nc.sync.dma_start(biascol_sb[:], d["biascol"].ap())
            nc.sync.dma_start(
                decT_sb[:], d["decT"].ap().rearrange("(o p) b -> p o b", p=P)
            )
            nc.vector.memset(bc_lhs[:], 0.0)
            nc.vector.memset(bc_lhs[0:1, :], 1.0)

            # ---- Q = dec @ W^T + bias  (also serves as PE warmup) ----
            for ci in range(CT):
                q_ps = psQ.tile([P, BL], F32, tag="q")
                for hi in range(HT):
                    nc.tensor.matmul(
                        q_ps[:],
                        WT_sb[:, hi, ci * P:(ci + 1) * P],
                        decT_sb[:, hi, :],
                        start=(hi == 0),
                        stop=(hi == HT - 1),
                    )
                nc.vector.tensor_tensor(
                    qb_sb[:, ci, :],
                    q_ps[:],
                    biascol_sb[:, ci, None].to_broadcast((P, BL)),
                    ALU.add,
                )

            # ---- per-batch pipeline ----
            for b in range(BL):
                encT_t = encp.tile([P, HT, T], BF, tag="encT")
                for hi in range(HT):
                    nc.sync.dma_start(
                        encT_t[:, hi, :],
                        d["encT"].ap()[b].rearrange("(o p) t -> p o t", p=P)[:, hi, :],
                    )
                ash_t = ashp.tile([P, T], BF, tag="ash")
                nc.vector.memset(ash_t[:], 0.0)
                nc.sync.dma_start(ash_t[0:3, :], d["ashift"].ap()[b])

                scores_row = rows.tile([1, T], F32, tag="scores")
                for ti in range(TT):
                    tsl = slice(ti * NT, (ti + 1) * NT)
                    for ci in range(CT):
                        csl = slice(ci * P, (ci + 1) * P)
                        e_ps = psE.tile([P, NT], F32, tag="e")
                        for hi in range(HT):
                            nc.tensor.matmul(
                                e_ps[:],
                                VT_sb[:, hi, csl],
                                encT_t[:, hi, tsl],
                                start=(hi == 0),
                                stop=False,
                            )
                        nc.tensor.matmul(
                            e_ps[:],
                            MT_sb[:, csl],
                            ash_t[:, tsl],
                            start=False,
                            stop=True,
                        )
                        wtanh = work.tile([P, NT], BF, tag="wtanh")
                        nc.scalar.activation(
                            wtanh[:], e_ps[:], AF.Tanh, bias=qb_sb[:, ci, b:b + 1]
                        )
                        s_ps = psS.tile([1, NT], F32, tag="s")
                        nc.tensor.matmul(
                            s_ps[:],
                            wre_sb[:, ci:ci + 1],
                            wtanh[:],
                            start=True,
                            stop=True,
                        )
                        if ci == 0:
                            nc.any.tensor_copy(scores_row[:, tsl], s_ps[:])
                        else:
                            nc.any.tensor_add(
                                scores_row[:, tsl], scores_row[:, tsl], s_ps[:]
                            )

                if _STAGE < 2:
                    nc.sync.dma_start(d["align_out"].ap()[b:b + 1, :], scores_row[:])
                    ctx_sb = work.tile([P, HT], F32, tag="ctx")
                    nc.vector.memset(ctx_sb[:], 0.0)
                    nc.sync.dma_start(
                        d["ctx_out"].ap().rearrange("b (o p) -> b p o", p=P)[b],
                        ctx_sb[:],
                    )
                    continue
                # ---- softmax over T (single-partition row, f32) ----
                mx = rows.tile([1, 1], F32, tag="mx")
                nc.vector.reduce_max(mx[:], scores_row[:], axis=AX.X)
                nmx = rows.tile([1, 1], F32, tag="nmx")
                if "tsp" in _DISABLED:
                    nc.scalar.activation(nmx[:], mx[:], AF.Copy, scale=-1.0)
                else:
                    nc.vector.tensor_scalar_mul(nmx[:], mx[:], -1.0)
                esc = rows.tile([1, T], F32, tag="esc")
                nc.scalar.activation(esc[:], scores_row[:], AF.Exp, bias=nmx[:])
                ssum = rows.tile([1, 1], F32, tag="ssum")
                nc.vector.reduce_sum(ssum[:], esc[:], axis=AX.X)
                align_row = rows.tile([1, T], F32, tag="alrow")
                if "recip" in _DISABLED:
                    nc.vector.tensor_tensor(
                        align_row[:], esc[:],
                        ssum[:].to_broadcast((1, T)), ALU.divide,
                    )
                else:
                    rsum = rows.tile([1, 1], F32, tag="rsum")
                    nc.vector.reciprocal(rsum[:], ssum[:])
                    nc.vector.tensor_scalar_mul(align_row[:], esc[:], rsum[:])
                nc.sync.dma_start(d["align_out"].ap()[b:b + 1, :], align_row[:])

                if _STAGE < 3:
                    ctx_sb = work.tile([P, HT], F32, tag="ctx")
                    nc.vector.memset(ctx_sb[:], 0.0)
                    nc.sync.dma_start(
                        d["ctx_out"].ap().rearrange("b (o p) -> b p o", p=P)[b],
                        ctx_sb[:],
                    )
                    continue
                # ---- broadcast alignment to 128 partitions (ones outer-product) ----
                align_pad = bcp.tile([P, T], BF, tag="alpad")
                nc.vector.memset(align_pad[:], 0.0)
                nc.any.tensor_copy(align_pad[0:1, :], align_row[:])
                align_bc = bcp.tile([P, T], BF, tag="albc")
                for ti in range(TT):
                    tsl = slice(ti * NT, (ti + 1) * NT)
                    b_ps = psB.tile([P, NT], F32, tag="b")
                    nc.tensor.matmul(
                        b_ps[:], bc_lhs[:], align_pad[:, tsl], start=True, stop=True
                    )
                    nc.any.tensor_copy(align_bc[:, tsl], b_ps[:])

                # ---- context[h] = sum_t enc[t,h] * align[t] ----
                ctx_sb = work.tile([P, HT], F32, tag="ctx")
                if _STAGE < 4:
                    nc.vector.memset(ctx_sb[:], 0.0)
                    nc.sync.dma_start(
                        d["ctx_out"].ap().rearrange("b (o p) -> b p o", p=P)[b],
                        ctx_sb[:],
                    )
                    continue
                scr = bcp.tile([P, T], BF, tag="scr")
                for hi in range(HT):
                    # tensor_tensor_reduce (fused) crashes on HW; use 2 DVE ops
                    nc.vector.tensor_tensor(
                        scr[:], encT_t[:, hi, :], align_bc[:], ALU.mult
                    )
                    nc.vector.reduce_sum(
                        ctx_sb[:, hi:hi + 1], scr[:], axis=AX.X
                    )
                nc.sync.dma_start(
                    d["ctx_out"].ap().rearrange("b (o p) -> b p o", p=P)[b],
                    ctx_sb[:],
                )


def _build(finalize=True):
    if "nc" in _CACHE:
        return _CACHE["nc"]
    nc = bacc.Bacc("TRN2", target_bir_lowering=False, debug=False, num_devices=NCORES)
    d = {}
    d["encT"] = nc.dram_tensor("encT", [BL, H, T], BF, kind="ExternalInput")
    d["ashift"] = nc.dram_tensor("ashift", [BL, 3, T], BF, kind="ExternalInput")
    d["decT"] = nc.dram_tensor("decT", [H, BL], BF, kind="ExternalInput")
    d["VT"] = nc.dram_tensor("VT", [H, CTX], BF, kind="ExternalInput")
    d["WT"] = nc.dram_tensor("WT", [H, CTX], BF, kind="ExternalInput")
    d["MT"] = nc.dram_tensor("MT", [3, CTX], BF, kind="ExternalInput")
    d["wre"] = nc.dram_tensor("wre", [P, CT], BF, kind="ExternalInput")
    d["biascol"] = nc.dram_tensor("biascol", [P, CT], F32, kind="ExternalInput")
    d["ctx_out"] = nc.dram_tensor("ctx_out", [BL, H], F32, kind="ExternalOutput")
    d["align_out"] = nc.dram_tensor("align_out", [BL, T], F32, kind="ExternalOutput")
    _emit(nc, d)
    if finalize:
        nc.finalize()
        _CACHE["nc"] = nc
    return nc


def _prep_in_maps(decoder_output, encoder_outputs, last_alignment, W, V, U, b, w,
                  conv_w, conv_b):
    f32 = np.float32
    dec = np.asarray(decoder_output, f32)[:, 0, :]          # (B,H)
    enc = np.asarray(encoder_outputs, f32)                  # (B,T,H)
    a = np.asarray(last_alignment, f32)                     # (B,T)
    W = np.asarray(W, f32); V = np.asarray(V, f32); U = np.asarray(U, f32)
    b = np.asarray(b, f32); w = np.asarray(w, f32)
    cw = np.asarray(conv_w, f32)[:, 0, :]                   # (32,3)
    cb = np.asarray(conv_b, f32)

    M = (U @ cw).astype(f32)                                # (CTX,3)
    bias_full = (b + U @ cb).astype(f32)                    # (CTX,)
    ap = np.pad(a, ((0, 0), (1, 1)))
    ashift = np.stack([ap[:, 0:T], ap[:, 1:T + 1], ap[:, 2:T + 2]], axis=1)  # (B,3,T)

    VT = np.ascontiguousarray(V.T).astype(BF_NP)
    WT = np.ascontiguousarray(W.T).astype(BF_NP)
    MT = np.ascontiguousarray(M.T).astype(BF_NP)            # (3,CTX)
    wre = np.ascontiguousarray(w.reshape(CT, P).T).astype(BF_NP)      # (P,CT)
    biascol = np.ascontiguousarray(bias_full.reshape(CT, P).T).astype(f32)

    in_maps = []
    for i in range(NCORES):
        bs = slice(i * BL, (i + 1) * BL)
        encT = np.ascontiguousarray(enc[bs].transpose(0, 2, 1)).astype(BF_NP)
        in_maps.append({
            "encT": encT,
            "ashift": ashift[bs].astype(BF_NP),
            "decT": np.ascontiguousarray(dec[bs].T).astype(BF_NP),
            "VT": VT, "WT": WT, "MT": MT, "wre": wre, "biascol": biascol,
        })
    return in_maps


def kernel(**inputs):
    nc = _build()
    in_maps = _prep_in_maps(**inputs)
    res = run_bass_kernel_spmd(nc, in_maps, core_ids=list(range(NCORES)))
    context = np.concatenate([r["ctx_out"] for r in res.results], axis=0)
    alignment = np.concatenate([r["align_out"] for r in res.results], axis=0)
    return context.astype(np.float32), alignment.astype(np.float32)
